# revision 1
# baseline (speedup 1.0000x reference)
"""NeuronMemory retrieval kernel v2 for 8 TRN2 NeuronCores.

Data-parallel over tokens (512/core). Per-core pipeline:
  B:  router softmax + Q = sum_n w_n (x @ W_n)  (fp32r PE, as v1)
      QT16 = fp16(Q^T * 1/sqrt(R))
  C:  scores = QT16^T @ KT16 (fp16 PE, 1-pass) -> PSUM f32
      scalar: PSUM -> sc16 (fp16 SBUF) -> DMA spill to DRAM rows of L=64
      DVE:    PSUM -> blockmax (tensor_reduce max, L=64) -> bm16
  L2: per (tile, half): top-8 blocks via max8/find_index8 on bm16 [128,256]
  G1: dma_gather 8 block-rows (64 f16) per token per half from the spill
  L3: per (tile, half): top-8 items of gathered [128,512] -> 16 cheap cands
      with exact global key indices
  SEL: top-12 of 16 by cheap value (max8 + match_replace + max8)
  G2: dma_gather 12 K rows (512B f32) per token; exact rescore on DVE
      (mult + segmented add) -> exact top-8 + softmax weights
  G3: dma_gather 8 V rows (2KB f16) per token; weighted accumulate -> out

Index layout for dma_gather: int16, idx j at (partition j%16, col j//16),
j = slot*128 + p so dst[p, slot, :] = src[idx[p, slot], :]. Built via a
3-hop DRAM reshuffle (8 tiny scatter DMAs), batched for all tiles.
"""
import copy

import numpy as np

import concourse.bacc as bacc
import concourse.bass as bass
import concourse.mybir as mybir
from concourse.tile import TileContext
from concourse.bass_utils import run_bass_kernel_spmd

P = 128
D_MODEL = 1024
RANK = 128
N_COMPRESS = 16
N_KNOWLEDGE = 32768
K_TOP = 8
B, S = 2, 2048
N_CORES = 8
TOK_PER_CORE = (B * S) // N_CORES      # 512
N_TILES = TOK_PER_CORE // P            # 4
N_DC = D_MODEL // P                    # 8
N_Q = 4                                # quarters of 8192
QW = N_KNOWLEDGE // N_Q
N_H = 2                                # halves of 16384
HW_ = N_KNOWLEDGE // N_H               # 16384
L = 128                                # spill block (256B f16 rows)
NBH = HW_ // L                         # 256 blocks per half
N_G = 4                                # neuron groups of 4 (B phase)
NCAND = 16                             # cheap candidates (8 per half)
NRESC = 12                             # exactly rescored candidates
SCALE = 1.0 / np.sqrt(np.float32(RANK))

f32 = mybir.dt.float32
f16 = mybir.dt.float16
i16 = mybir.dt.int16
u32 = mybir.dt.uint32


def _wrap_indices(nc, pool, name, src_f32_ap, nslots, scr_a, scr_b):
    """[128, nslots] f32 row indices -> wrapped int16 [128, nslots*8] tile
    (only partitions 0..15 meaningful). Requires nslots % 1 == 0."""
    src16 = pool.tile([P, nslots], i16, name=f"wi_{name}_src")
    nc.vector.tensor_copy(out=src16[:], in_=src_f32_ap)
    nc.sync.dma_start(out=scr_a[:, :nslots], in_=src16[:])
    for p8 in range(8):
        ina = copy.copy(scr_a[:, :])
        ina.offset = 64 * 16 * p8         # scr_a row stride 64, 16 rows per p8 group
        ina.ap = mybir.VecI64Pair([[64, 16], [1, nslots]])
        outa = copy.copy(scr_b[:])
        outa.offset = p8
        outa.ap = mybir.VecI64Pair([[8 * nslots, 16], [8, nslots]])
        with nc.allow_non_contiguous_dma(reason="tiny idx wrap"):
            nc.sync.dma_start(out=outa, in_=ina)
    w = pool.tile([P, nslots * 8], i16, name=f"wi_{name}_w")
    nc.gpsimd.memset(w[:], 0)
    inb = copy.copy(scr_b[:])
    inb.ap = mybir.VecI64Pair([[8 * nslots, 16], [1, 8 * nslots]])
    nc.sync.dma_start(out=w[:16, :], in_=inb)
    return w


def _build(dbg=False):
    nc = bacc.Bacc("TRN2", target_bir_lowering=False, debug=False, num_devices=N_CORES)

    xT = nc.declare_dram_parameter("xT", [P, N_DC * TOK_PER_CORE], f32, isOutput=False)
    rw = nc.declare_dram_parameter("rw", [P, N_DC * N_COMPRESS], f32, isOutput=False)
    Wg = nc.declare_dram_parameter("Wg", [N_G * N_DC * P, 512], f32, isOutput=False)
    KT16 = nc.declare_dram_parameter("KT16", [P, N_KNOWLEDGE], f16, isOutput=False)
    KD = nc.declare_dram_parameter("KD", [N_KNOWLEDGE, RANK], f32, isOutput=False)
    VD16 = nc.declare_dram_parameter("VD16", [N_KNOWLEDGE, D_MODEL], f16, isOutput=False)
    ident = nc.declare_dram_parameter("ident", [P, P], f32, isOutput=False)
    iotaP = nc.declare_dram_parameter("iotaP", [P, 1], f32, isOutput=False)
    io8 = nc.declare_dram_parameter("io8", [P, NCAND], f32, isOutput=False)  # 0..15 bcast
    out = nc.declare_dram_parameter("out", [TOK_PER_CORE, D_MODEL], f32, isOutput=True)
    if dbg:
        d_q = nc.declare_dram_parameter("d_q", [P, N_TILES * RANK], f32, isOutput=True)
        d_bm = nc.declare_dram_parameter("d_bm", [P, N_TILES * N_H * NBH], f32, isOutput=True)
        d_cv = nc.declare_dram_parameter("d_cv", [P, N_TILES * NCAND], f32, isOutput=True)
        d_ci = nc.declare_dram_parameter("d_ci", [P, N_TILES * NCAND], f32, isOutput=True)
        d_rs = nc.declare_dram_parameter("d_rs", [P, N_TILES * NRESC], f32, isOutput=True)
        d_vg = nc.declare_dram_parameter("d_vg", [P, N_TILES * K_TOP], f32, isOutput=True)
        d_w8 = nc.declare_dram_parameter("d_w8", [P, N_TILES * K_TOP], f32, isOutput=True)

    # internal DRAM
    scSp = {(t, h): nc.dram_tensor(f"scSp_{t}_{h}", [HW_ // L * P, L], f16)
            for t in range(N_TILES) for h in range(N_H)}

    Wg_v = Wg.rearrange("(g dc p) n -> g dc p n", g=N_G, dc=N_DC)

    with TileContext(nc) as tc:
        with (
            tc.tile_pool(name="const", bufs=1) as cpool,
            tc.tile_pool(name="kt", bufs=2) as ktpool,
            tc.tile_pool(name="sc", bufs=3) as scpool,
            tc.tile_pool(name="wld", bufs=3) as wpool,
            tc.tile_pool(name="gat1", bufs=2) as g1pool,
            tc.tile_pool(name="gat2", bufs=2) as g2pool,
            tc.tile_pool(name="gat3", bufs=2) as g3pool,
            tc.tile_pool(name="acc", bufs=2) as apool,
            tc.tile_pool(name="small", bufs=6) as spool,
        ):
            # ---- persistent loads ----
            xT_sb = cpool.tile([P, N_DC * TOK_PER_CORE], f32)
            rw_sb = cpool.tile([P, N_DC * N_COMPRESS], f32)
            id_sb = cpool.tile([P, P], f32)
            iota_sb = cpool.tile([P, 1], f32)
            io8_sb = cpool.tile([P, NCAND], f32)
            nc.sync.dma_start(out=xT_sb[:], in_=xT[:])
            nc.sync.dma_start(out=rw_sb[:], in_=rw[:])
            nc.sync.dma_start(out=id_sb[:], in_=ident[:])
            nc.sync.dma_start(out=iota_sb[:], in_=iotaP[:])
            nc.sync.dma_start(out=io8_sb[:], in_=io8[:])

            wts_sb = cpool.tile([P, N_TILES * N_COMPRESS], f32)
            Q_sb = cpool.tile([P, N_TILES * RANK], f32)
            QT16_sb = cpool.tile([P, N_TILES * P], f16)
            bm16_sb = cpool.tile([P, N_TILES * N_H * NBH], f16)   # 2048 f16/part
            cv_sb = cpool.tile([P, N_TILES * NCAND], f16)
            ci_sb = cpool.tile([P, N_TILES * NCAND], f32)
            sgidx_sb = cpool.tile([P, N_TILES * N_H * 8], f32)    # spill-gather rows
            kg_sb = cpool.tile([P, N_TILES * NRESC], f32)         # K-gather rows (= key idx)
            vg_sb = cpool.tile([P, N_TILES * K_TOP], f32)         # V-gather rows
            w8_sb = cpool.tile([P, N_TILES * K_TOP], f32)         # softmax weights

            def tok(t):
                return slice(t * P, (t + 1) * P)

            psy_cm = tc.tile_pool(name="ps_y", bufs=2, space="PSUM")
            psy = psy_cm.__enter__()
            pss_cm = tc.tile_pool(name="ps_small", bufs=2, space="PSUM")
            pss = pss_cm.__enter__()
            psb_cm = tc.tile_pool(name="ps_big", bufs=2, space="PSUM")
            psb = psb_cm.__enter__()

            # ================= A: router softmax =================
            for t in range(N_TILES):
                rps_full = pss.tile([P, P], f32, space="PSUM", tag="tps", name=f"rps_{t}")
                rps = rps_full[:, :N_COMPRESS]
                for dc in range(N_DC):
                    nc.tensor.matmul(
                        out=rps,
                        lhsT=xT_sb[:, dc * TOK_PER_CORE + t * P:dc * TOK_PER_CORE + (t + 1) * P],
                        rhs=rw_sb[:, dc * N_COMPRESS:(dc + 1) * N_COMPRESS],
                        start=(dc == 0), stop=(dc == N_DC - 1),
                    )
                w = wts_sb[:, t * N_COMPRESS:(t + 1) * N_COMPRESS]
                mx = spool.tile([P, 1], f32, tag="mx")
                sm = spool.tile([P, 1], f32, tag="sm")
                ex = spool.tile([P, N_COMPRESS], f32, tag="ex")
                nc.vector.tensor_reduce(out=mx[:], in_=rps, op=mybir.AluOpType.max, axis=mybir.AxisListType.X)
                nc.vector.tensor_scalar(out=ex[:], in0=rps, scalar1=mx[:, :1], scalar2=None, op0=mybir.AluOpType.subtract)
                nc.scalar.activation(out=ex[:], in_=ex[:], func=mybir.ActivationFunctionType.Exp,
                                     accum_out=sm[:, :1])
                rcp = spool.tile([P, 1], f32, tag="rcp")
                nc.vector.reciprocal(out=rcp[:], in_=sm[:, :1])
                nc.vector.tensor_scalar(out=w, in0=ex[:], scalar1=rcp[:, :1], scalar2=None, op0=mybir.AluOpType.mult)

            # full KT16 resident in SBUF (64KB/part)
            ktq_full = cpool.tile([P, N_KNOWLEDGE], f16)
            for q in range(N_Q):
                nc.sync.dma_start(out=ktq_full[:, q * QW:(q + 1) * QW], in_=KT16[:, q * QW:(q + 1) * QW])

            def emit_B(tiles):
                yps_tiles = {}
                for g in range(N_G):
                    for dc in range(N_DC):
                        wtile = wpool.tile([P, 512], f32, tag="wld", name=f"w_{tiles[0]}_{g}_{dc}")
                        nc.sync.dma_start(out=wtile[:], in_=Wg_v[g, dc])
                        for t in tiles:
                            if dc == 0:
                                yps_tiles[t] = psy.tile([P, 512], f32, space="PSUM", tag="ps",
                                                        name=f"yps_{tiles[0]}_{g}_{t}")
                            nc.tensor.matmul(
                                out=yps_tiles[t][:],
                                lhsT=xT_sb[:, dc * TOK_PER_CORE + t * P:dc * TOK_PER_CORE + (t + 1) * P],
                                rhs=wtile[:],
                                start=(dc == 0), stop=(dc == N_DC - 1),
                            )
                    for t in tiles:
                        q_ = Q_sb[:, t * RANK:(t + 1) * RANK]
                        for n in range(4):
                            ncomp = g * 4 + n
                            wcol = wts_sb[:, t * N_COMPRESS + ncomp:t * N_COMPRESS + ncomp + 1]
                            ypart = yps_tiles[t][:, n * RANK:(n + 1) * RANK]
                            if g == 0 and n == 0:
                                nc.vector.tensor_scalar(out=q_, in0=ypart, scalar1=wcol, scalar2=None,
                                                        op0=mybir.AluOpType.mult)
                            else:
                                nc.vector.scalar_tensor_tensor(out=q_, in0=ypart, scalar=wcol, in1=q_,
                                                               op0=mybir.AluOpType.mult,
                                                               op1=mybir.AluOpType.add)
                for t in tiles:
                    tps = pss.tile([P, P], f32, space="PSUM", tag="tps", name=f"tps_{t}")
                    nc.tensor.transpose(out=tps[:], in_=Q_sb[:, t * RANK:(t + 1) * RANK], identity=id_sb[:])
                    nc.scalar.activation(out=QT16_sb[:, tok(t)], in_=tps[:],
                                         func=mybir.ActivationFunctionType.Copy, scale=float(SCALE))


            sgidx_u = cpool.tile([P, N_TILES * N_H * 8], u32)
            gat1 = {}

            def emit_half(t, h):
                # L2: top-8 blocks
                bmv = bm16_sb[:, (t * N_H + h) * NBH:(t * N_H + h + 1) * NBH]
                bv8 = spool.tile([P, 8], f16, tag="bv8", name=f"bv8_{t}_{h}")
                bu8 = spool.tile([P, 8], u32, tag="bu8", name=f"bu8_{t}_{h}")
                nc.vector.max(out=bv8[:], in_=bmv)
                nc.vector.max_index(out=bu8[:], in_max=bv8[:], in_values=bmv)
                bi8f = spool.tile([P, 8], f32, tag="bi8f", name=f"bi8f_{t}_{h}")
                nc.vector.tensor_copy(out=bi8f[:], in_=bu8[:])
                so = sgidx_sb[:, (t * N_H + h) * 8:(t * N_H + h + 1) * 8]
                rowbase = spool.tile([P, 1], f32, tag="rowbase", name=f"rb_{t}_{h}")
                nc.vector.tensor_scalar(out=rowbase[:], in0=iota_sb[:], scalar1=float(NBH),
                                        scalar2=None, op0=mybir.AluOpType.mult)
                nc.vector.tensor_scalar(out=so, in0=bi8f[:], scalar1=rowbase[:, :1],
                                        scalar2=None, op0=mybir.AluOpType.add)
                # G1: gather the 8 winning block rows
                su = sgidx_u[:, (t * N_H + h) * 8:(t * N_H + h + 1) * 8]
                nc.vector.tensor_copy(out=su, in_=so)
                g1 = g1pool.tile([P, 8 * L], f16, tag="g1", name=f"g1_{t}_{h}")
                for sslot in range(8):
                    nc.gpsimd.indirect_dma_start(
                        out=g1[:, sslot * L:(sslot + 1) * L],
                        out_offset=None,
                        in_=scSp[(t, h)][:],
                        in_offset=bass.IndirectOffsetOnAxis(
                            ap=sgidx_u[:, (t * N_H + h) * 8 + sslot:(t * N_H + h) * 8 + sslot + 1],
                            axis=0),
                    )
                gat1[(t, h)] = g1
                # L3: top-8 items of the gathered 1024
                iv8 = cv_sb[:, t * NCAND + h * 8:t * NCAND + (h + 1) * 8]
                ip8 = spool.tile([P, 8], u32, tag="ip8", name=f"ip8_{t}_{h}")
                nc.vector.max(out=iv8, in_=g1[:])
                nc.vector.max_index(out=ip8[:], in_max=iv8, in_values=g1[:])
                slot_u = spool.tile([P, 8], u32, tag="slot_u", name=f"slu_{t}_{h}")
                nc.vector.tensor_scalar(out=slot_u[:], in0=ip8[:], scalar1=7, scalar2=None,
                                        op0=mybir.AluOpType.logical_shift_right)
                slotf = spool.tile([P, 8], f32, tag="slotf", name=f"slf_{t}_{h}")
                ip8f = spool.tile([P, 8], f32, tag="ip8f", name=f"ipf_{t}_{h}")
                nc.vector.tensor_copy(out=slotf[:], in_=slot_u[:])
                nc.vector.tensor_copy(out=ip8f[:], in_=ip8[:])
                off8 = spool.tile([P, 8], f32, tag="off8", name=f"off_{t}_{h}")
                nc.vector.scalar_tensor_tensor(out=off8[:], in0=slotf[:], scalar=-float(L),
                                               in1=ip8f[:], op0=mybir.AluOpType.mult,
                                               op1=mybir.AluOpType.add)
                blkf = spool.tile([P, 8], f32, tag="blkf", name=f"blk_{t}_{h}")
                rowb2 = spool.tile([P, 1], f32, tag="rowbase2", name=f"rb2_{t}_{h}")
                nc.vector.tensor_scalar(out=rowb2[:], in0=iota_sb[:], scalar1=-float(NBH),
                                        scalar2=None, op0=mybir.AluOpType.mult)
                nc.vector.tensor_scalar(out=blkf[:], in0=so, scalar1=rowb2[:, :1], scalar2=None,
                                        op0=mybir.AluOpType.add)
                gblk = spool.tile([P, 8], f32, tag="gblk", name=f"gbk_{t}_{h}")
                junk = spool.tile([P, 8], f32, tag="junk8", name=f"jk8_{t}_{h}")
                for j in range(8):
                    nc.vector.scalar_tensor_tensor(
                        out=junk[:], in0=io8_sb[:, :8], scalar=slotf[:, j:j + 1], in1=blkf[:],
                        op0=mybir.AluOpType.is_equal, op1=mybir.AluOpType.mult,
                        accum_out=gblk[:, j:j + 1])
                gi = ci_sb[:, t * NCAND + h * 8:t * NCAND + (h + 1) * 8]
                nc.vector.scalar_tensor_tensor(out=gi, in0=gblk[:], scalar=float(L),
                                               in1=off8[:], op0=mybir.AluOpType.mult,
                                               op1=mybir.AluOpType.add)
                if h == 1:
                    nc.vector.tensor_scalar(out=gi, in0=gi, scalar1=float(HW_), scalar2=None,
                                            op0=mybir.AluOpType.add)

            def emit_C(tiles):
                for q in range(N_Q):
                    h = q // 2
                    qh = q % 2
                    for t in tiles:
                        for cc in range(8):                  # 1024-wide psum windows
                            cps = psb.tile([P, 1024], f32, space="PSUM", tag="cps")
                            for j in range(2):
                                nc.tensor.matmul(
                                    out=cps[:, j * 512:(j + 1) * 512],
                                    lhsT=QT16_sb[:, tok(t)],
                                    rhs=ktq_full[:, q * QW + cc * 1024 + j * 512:q * QW + cc * 1024 + (j + 1) * 512],
                                    start=True, stop=True,
                                )
                            sc16c = scpool.tile([P, 1024], f16, tag="sc16c")
                            nc.scalar.copy(out=sc16c[:], in_=cps[:])
                            boff = t * N_H * NBH + h * NBH + qh * 64 + cc * 8
                            nc.vector.tensor_reduce(
                                out=bm16_sb[:, boff:boff + 8],
                                in_=cps.rearrange("p (b l) -> p b l", l=L),
                                op=mybir.AluOpType.max, axis=mybir.AxisListType.X)
                            o_sp = copy.copy(scSp[(t, h)][:])
                            o_sp.offset = (qh * 64 + cc * 8) * L
                            o_sp.ap = mybir.VecI64Pair([[NBH * L, P], [1, 1024]])
                            nc.sync.dma_start(out=o_sp, in_=sc16c[:])
                    if qh == 1:
                        for te in tiles:
                            emit_half(te, h)

            emit_B((0, 1))
            emit_C((0, 1))
            emit_B((2, 3))
            emit_C((2, 3))
            if dbg:
                nc.sync.dma_start(out=d_q[:], in_=Q_sb[:])
                bmf = cpool.tile([P, N_TILES * N_H * NBH], f32)
                nc.vector.tensor_copy(out=bmf[:], in_=bm16_sb[:])
                nc.sync.dma_start(out=d_bm[:], in_=bmf[:])

            psb_cm.__exit__(None, None, None)
            pss_cm.__exit__(None, None, None)
            psy_cm.__exit__(None, None, None)

            if dbg:
                cvf = cpool.tile([P, N_TILES * NCAND], f32)
                nc.vector.tensor_copy(out=cvf[:], in_=cv_sb[:])
                nc.sync.dma_start(out=d_cv[:], in_=cvf[:])
                nc.sync.dma_start(out=d_ci[:], in_=ci_sb[:])

            # ================= SEL: top-12 of 16 cheap, resolve K rows =================
            for t in range(N_TILES):
                cv = cv_sb[:, t * NCAND:(t + 1) * NCAND]
                ci = ci_sb[:, t * NCAND:(t + 1) * NCAND]
                s8a = spool.tile([P, 8], f16, tag="s8a")
                pa = spool.tile([P, 8], u32, tag="pa")
                nc.vector.max(out=s8a[:], in_=cv)
                nc.vector.max_index(out=pa[:], in_max=s8a[:], in_values=cv)
                cvb = spool.tile([P, NCAND], f16, tag="cvb")
                nc.vector.match_replace(out=cvb[:], in_to_replace=s8a[:], in_values=cv,
                                        imm_value=-60000.0)
                s8b = spool.tile([P, 8], f16, tag="s8b")
                pb = spool.tile([P, 8], u32, tag="pb")
                nc.vector.max(out=s8b[:], in_=cvb[:])
                nc.vector.max_index(out=pb[:], in_max=s8b[:], in_values=cvb[:])
                # slots of the 12: pa[0..8), pb[0..4)
                paf = spool.tile([P, NRESC], f32, tag="paf")
                nc.vector.tensor_copy(out=paf[:, :8], in_=pa[:])
                nc.vector.tensor_copy(out=paf[:, 8:12], in_=pb[:, :4])
                # kg[j] = ci[slot_j]
                kg = kg_sb[:, t * NRESC:(t + 1) * NRESC]
                junk = spool.tile([P, NCAND], f32, tag="junk16")
                for j in range(NRESC):
                    nc.vector.scalar_tensor_tensor(
                        out=junk[:], in0=io8_sb[:], scalar=paf[:, j:j + 1], in1=ci,
                        op0=mybir.AluOpType.is_equal, op1=mybir.AluOpType.mult,
                        accum_out=kg[:, j:j + 1])

            # ================= G2 + rescore =================
            kg_u = cpool.tile([P, N_TILES * NRESC], u32)
            for t in range(N_TILES):
                nc.vector.tensor_copy(out=kg_u[:, t * NRESC:(t + 1) * NRESC],
                                      in_=kg_sb[:, t * NRESC:(t + 1) * NRESC])
                g2 = g2pool.tile([P, NRESC * RANK], f32, tag="g2")
                for sslot in range(NRESC):
                    nc.gpsimd.indirect_dma_start(
                        out=g2[:, sslot * RANK:(sslot + 1) * RANK],
                        out_offset=None,
                        in_=KD[:],
                        in_offset=bass.IndirectOffsetOnAxis(
                            ap=kg_u[:, t * NRESC + sslot:t * NRESC + sslot + 1], axis=0),
                    )
                # prod = g2 * Q (broadcast Q over the 12 candidate slots)
                qb = copy.copy(Q_sb[:, t * RANK:(t + 1) * RANK])
                qb.ap = mybir.VecI64Pair([[qb.ap[0][0], P], [0, NRESC], [1, RANK]])
                prod = scpool.tile([P, NRESC * RANK], f32, tag="prod")
                nc.vector.tensor_tensor(out=prod.rearrange("p (s e) -> p s e", s=NRESC),
                                        in0=g2.rearrange("p (s e) -> p s e", s=NRESC),
                                        in1=qb, op=mybir.AluOpType.mult)
                rsc = spool.tile([P, NRESC], f32, tag="rsc")
                nc.vector.tensor_reduce(out=rsc[:], in_=prod.rearrange("p (s e) -> p s e", s=NRESC),
                                        op=mybir.AluOpType.add, axis=mybir.AxisListType.X)
                nc.vector.tensor_scalar(out=rsc[:], in0=rsc[:], scalar1=float(SCALE), scalar2=None,
                                        op0=mybir.AluOpType.mult)
                if dbg:
                    nc.sync.dma_start(out=d_rs[:, t * NRESC:(t + 1) * NRESC], in_=rsc[:])
                # exact top-8
                v8 = spool.tile([P, 8], f32, tag="v8")
                p8i = spool.tile([P, 8], u32, tag="p8i")
                nc.vector.max(out=v8[:], in_=rsc[:])
                nc.vector.max_index(out=p8i[:], in_max=v8[:], in_values=rsc[:])
                p8f = spool.tile([P, 8], f32, tag="p8f")
                nc.vector.tensor_copy(out=p8f[:], in_=p8i[:])
                # vg[j] = kg[slot_j]
                vg = vg_sb[:, t * K_TOP:(t + 1) * K_TOP]
                junk = spool.tile([P, NRESC], f32, tag="junk12")
                for j in range(K_TOP):
                    nc.vector.scalar_tensor_tensor(
                        out=junk[:], in0=io8_sb[:, :NRESC], scalar=p8f[:, j:j + 1],
                        in1=kg_sb[:, t * NRESC:(t + 1) * NRESC],
                        op0=mybir.AluOpType.is_equal, op1=mybir.AluOpType.mult,
                        accum_out=vg[:, j:j + 1])
                # softmax over v8
                w8 = w8_sb[:, t * K_TOP:(t + 1) * K_TOP]
                sm8 = spool.tile([P, 1], f32, tag="sm8")
                nc.vector.tensor_scalar(out=w8, in0=v8[:], scalar1=v8[:, :1], scalar2=None,
                                        op0=mybir.AluOpType.subtract)
                nc.scalar.activation(out=w8, in_=w8, func=mybir.ActivationFunctionType.Exp,
                                     accum_out=sm8[:, :1])
                rcp8 = spool.tile([P, 1], f32, tag="rcp8")
                nc.vector.reciprocal(out=rcp8[:], in_=sm8[:, :1])
                nc.vector.tensor_scalar(out=w8, in0=w8, scalar1=rcp8[:, :1], scalar2=None,
                                        op0=mybir.AluOpType.mult)
                if dbg:
                    nc.sync.dma_start(out=d_vg[:, t * 8:(t + 1) * 8], in_=vg)
                    nc.sync.dma_start(out=d_w8[:, t * 8:(t + 1) * 8], in_=w8)

            # ================= G3 + weighted accumulate =================
            vg_u = cpool.tile([P, N_TILES * K_TOP], u32)
            for t in range(N_TILES):
                nc.vector.tensor_copy(out=vg_u[:, t * K_TOP:(t + 1) * K_TOP],
                                      in_=vg_sb[:, t * K_TOP:(t + 1) * K_TOP])
                g3 = g3pool.tile([P, K_TOP * D_MODEL], f16, tag="g3")
                for sslot in range(K_TOP):
                    nc.gpsimd.indirect_dma_start(
                        out=g3[:, sslot * D_MODEL:(sslot + 1) * D_MODEL],
                        out_offset=None,
                        in_=VD16[:],
                        in_offset=bass.IndirectOffsetOnAxis(
                            ap=vg_u[:, t * K_TOP + sslot:t * K_TOP + sslot + 1], axis=0),
                    )
                acc = apool.tile([P, D_MODEL], f32, tag="acc")
                w8 = w8_sb[:, t * K_TOP:(t + 1) * K_TOP]
                nc.vector.tensor_scalar(out=acc[:], in0=g3[:, 0:D_MODEL], scalar1=w8[:, 0:1],
                                        scalar2=None, op0=mybir.AluOpType.mult)
                for j in range(1, K_TOP):
                    nc.vector.scalar_tensor_tensor(
                        out=acc[:], in0=g3[:, j * D_MODEL:(j + 1) * D_MODEL], scalar=w8[:, j:j + 1],
                        in1=acc[:], op0=mybir.AluOpType.mult, op1=mybir.AluOpType.add)
                nc.sync.dma_start(out=out[t * P:(t + 1) * P, :], in_=acc[:])

    nc.compile()
    return nc


_NC_CACHE = {}


def _get_nc(dbg=False):
    if dbg not in _NC_CACHE:
        _NC_CACHE[dbg] = _build(dbg)
    return _NC_CACHE[dbg]


def _prep_in_maps(x, router_w, compress_neurons, knowledge_K, knowledge_V):
    x = np.asarray(x, dtype=np.float32).reshape(B * S, D_MODEL)
    rwT = np.ascontiguousarray(np.asarray(router_w, dtype=np.float32).T)
    rw_r = np.ascontiguousarray(
        rwT.reshape(N_DC, P, N_COMPRESS).transpose(1, 0, 2).reshape(P, N_DC * N_COMPRESS))
    cn = np.asarray(compress_neurons, dtype=np.float32)
    Wg = np.ascontiguousarray(
        cn.reshape(N_G, 4, N_DC, P, RANK).transpose(0, 2, 3, 1, 4).reshape(N_G * N_DC * P, 4 * RANK))
    K = np.asarray(knowledge_K, dtype=np.float32)
    KT16 = np.ascontiguousarray(K.T.astype(np.float16))
    V16 = np.ascontiguousarray(np.asarray(knowledge_V, dtype=np.float32).astype(np.float16))
    ident = np.eye(P, dtype=np.float32)
    iotaP = np.arange(P, dtype=np.float32).reshape(P, 1)
    io8 = np.broadcast_to(np.arange(NCAND, dtype=np.float32), (P, NCAND)).copy()

    in_maps = []
    for c in range(N_CORES):
        xs = x[c * TOK_PER_CORE:(c + 1) * TOK_PER_CORE]
        xT = np.ascontiguousarray(
            xs.T.reshape(N_DC, P, TOK_PER_CORE).transpose(1, 0, 2).reshape(P, N_DC * TOK_PER_CORE))
        in_maps.append(dict(xT=xT, rw=rw_r, Wg=Wg, KT16=KT16, KD=K, VD16=V16,
                            ident=ident, iotaP=iotaP, io8=io8))
    return in_maps


def _ensure_ntff_hook():
    import sys as _sys
    import types as _types
    if "antenv.axon_hooks" in _sys.modules:
        return
    try:
        import antenv.axon_hooks  # noqa: F401
        return
    except ImportError:
        pass
    mod = _types.ModuleType("antenv.axon_hooks")
    _state = {"hook": None}
    mod.set_axon_ntff_profile_hook = lambda h: _state.__setitem__("hook", h)
    mod.get_axon_ntff_profile_hook = lambda: _state["hook"]
    _sys.modules["antenv.axon_hooks"] = mod
    try:
        from trn_agent_boot.trn_boot import _ntff_profile_via_ctypes
        mod.set_axon_ntff_profile_hook(_ntff_profile_via_ctypes("/opt/axon/libaxon_pjrt.so"))
    except Exception:
        pass


def _run(inputs, trace=False, dbg=False):
    if trace:
        _ensure_ntff_hook()
    nc = _get_nc(dbg)
    in_maps = _prep_in_maps(**inputs)
    res = run_bass_kernel_spmd(nc, in_maps, core_ids=list(range(N_CORES)), trace=trace)
    out = np.concatenate([res.results[c]["out"] for c in range(N_CORES)], axis=0)
    return out.reshape(B, S, D_MODEL), res


def kernel(x, router_w, compress_neurons, knowledge_K, knowledge_V):
    out, _ = _run(dict(x=x, router_w=router_w, compress_neurons=compress_neurons,
                       knowledge_K=knowledge_K, knowledge_V=knowledge_V))
    return out

